# revision 1
# baseline (speedup 1.0000x reference)
"""Trainium2 Bass kernel for nn_ClassLogitContrastiveLoss.

loss = mean_{bl,n}( sim[n, argmax_m d(n,m)] - sim[n, argmin_{m!=n} d(n,m)] )
with sim = yp @ yp^T (J=128 logits per point), d = pairwise euclidean dist
of the xyz points. B,L,J,N = 8,32,128,512.

Sharding: data-parallel over the fused B*L=256 batch dim, 32 items per core
on 8 NeuronCores (SPMD, no collectives); host sums the 8 partial outputs.

Per batch item (N=512 points, processed in 4 chunks of 128 PSUM rows):
  - argmax/argmin of dist == argmax/argmin of e[n,m] = sq[m] - 2<x_n,x_m>
    (the +sq[n] row term is constant over m; sqrt is monotone).
  - eb = -e is computed on the PE as a K=21 bf16 matmul: each fp32 operand
    is split into 3 bf16 parts (hi/mid/lo) and the 6 significant part-pairs
    per coordinate (+3 rows carrying sq[m] against an all-ones lhsT) are
    accumulated in fp32 PSUM -> ~1e-6 relative accuracy at full PE rate.
  - phase 1: DVE row-min of eb gives -rowmax(e); ScalarE emits the one-hot
    mask smax = Sign(e - rowmax) in {0 @argmax, -1 else}.
  - phase 2: the PE accumulates -BIG onto the diagonal of the SAME bank
    (excludes self-distance; "stop" flags are sim-only bookkeeping, so
    accumulating onto a closed group is legal on HW); DVE row-max then
    gives -rowmin_masked(e) and ScalarE emits
    tmin_neg = Sign(rowmin - e) in {0 @argmin, +1 else}.
  - The gather-and-subtract collapses into PE matmuls: with
    W = smax + tmin_neg (= onehot_max - onehot_min elementwise),
      sum_m sim .* W  ==  <U, ypred_native>,  U = ypT^T @ W,
    accumulated over chunks into one PSUM bank with a shared stationary ypT.
  - final dot: DVE multiplies U by native ypred; ScalarE's accumulate
    output reduces it into the per-batch column of the accumulator.
Scheduling: phase2(c) is emitted after phase1(c+1) (the two phases of one
chunk are a 6-deep cross-engine chain - skewing them keeps every engine
busy), and each batch's final dot is deferred into the next batch's chunk
loop.  DVE and ScalarE are the co-bottlenecks at ~1 elem/lane/cycle over
the 2 x 512^2 mask elements per item; PE streams ~7k matmul rows per item.
"""

import numpy as np
import ml_dtypes

BF16 = ml_dtypes.bfloat16
B, L, J, N = 8, 32, 128, 512
BL = B * L
NCORES = 8
PC = BL // NCORES          # 256/8 = 32 fused-batch items per core
NCHUNK = N // 128          # 4 partition chunks of the N=512 points
NPAIR = 21                 # 3 dims * 6 split-pairs + 3 sq rows
BIG = 32768.0              # 2^15, exact in bf16, >> any |e|

_CACHE = {}


def _build_nc(repeats=1):
    """Build (once) the single-core Bass/Tile program shared by all 8 cores.

    repeats>1 wraps the whole workload in a hardware For loop — used only for
    differential wall-clock benchmarking (amortizes dispatch overhead)."""
    key = ("nc", repeats)
    if key in _CACHE:
        return _CACHE[key]

    import concourse.bacc as bacc
    import concourse.tile as tile
    import concourse.mybir as mybir

    f32 = mybir.dt.float32
    bf16 = mybir.dt.bfloat16
    i32 = mybir.dt.int32
    AF = mybir.ActivationFunctionType
    ALU = mybir.AluOpType
    AX = mybir.AxisListType

    nc = bacc.Bacc(
        "TRN2",
        target_bir_lowering=False,
        debug=False,
        num_devices=NCORES,
    )

    ypn_d = nc.dram_tensor("ypn", [PC, J, N], f32, kind="ExternalInput").ap()
    ypt_d = nc.dram_tensor("ypt", [PC, 128, NCHUNK, J], bf16, kind="ExternalInput").ap()
    lr_d = nc.dram_tensor("lr", [PC, NPAIR, 2, N], bf16, kind="ExternalInput").ap()
    out_d = nc.dram_tensor("out", [128, 2 * PC], f32, kind="ExternalOutput").ap()

    with tile.TileContext(nc) as tc:
        with (
            tc.tile_pool(name="singles", bufs=1) as singles,
            tc.tile_pool(name="io", bufs=6) as io,
            tc.tile_pool(name="masks", bufs=12) as masks,
            tc.tile_pool(name="small", bufs=12) as small,
            tc.tile_pool(name="pef", bufs=5, space="PSUM") as pef,
            tc.tile_pool(name="pu", bufs=3, space="PSUM") as pu,
        ):
            # Constants: identity and BIG*identity (bf16) for the diagonal mask.
            iot = singles.tile([128, 128], i32)
            nc.gpsimd.iota(iot, pattern=[[1, 128]], base=0, channel_multiplier=-1)
            # Warm the ACT Sign table at t=0 so the ~2.7us table load overlaps
            # the first batch's DMAs instead of stalling the first real Sign.
            warm = singles.tile([1, 1], f32)
            nc.vector.memset(warm, 0.0)
            warm2 = singles.tile([1, 1], f32)
            nc.scalar.activation(out=warm2, in_=warm, func=AF.Sign)
            ident = singles.tile([128, 128], bf16)
            nc.vector.tensor_scalar(
                out=ident, in0=iot, scalar1=0, scalar2=None, op0=ALU.is_equal
            )
            neg_big_i = singles.tile([128, 128], bf16)
            nc.vector.tensor_scalar(
                out=neg_big_i, in0=iot, scalar1=0, scalar2=-BIG,
                op0=ALU.is_equal, op1=ALU.mult,
            )
            # Per-(partition, batch) partial sums of the loss numerator.
            # Separate tiles per writing engine so the DVE-reduce and the
            # ACT-accum writes never serialize against each other.
            accs_v = singles.tile([128, PC], f32)
            accs_a = singles.tile([128, PC], f32)
            nc.vector.memset(accs_v, 0.0)
            nc.gpsimd.memset(accs_a, 0.0)

            def final_dot(b, u_ps, ypn):
                # accs[:, b] = sum_m U[j, m] * ypred[j, m]
                # (tensor_tensor_reduce is a custom DVE op that does not run
                #  on this runtime path; mult on DVE, then the free-axis sum
                #  alternates between ScalarE's accumulate output and a DVE
                #  reduce to balance the two engines' load)
                scr = masks.tile([J, N], f32, tag="scr")
                nc.vector.tensor_tensor(out=scr, in0=u_ps, in1=ypn, op=ALU.mult)
                scr2 = masks.tile([J, N], f32, tag="scr2")
                nc.scalar.activation(
                    out=scr2, in_=scr, func=AF.Copy,
                    accum_out=accs_a[:, b:b + 1],
                )

            import contextlib

            loop_cm = (
                tc.For_i(0, repeats, 1) if repeats > 1 else contextlib.nullcontext()
            )
            with loop_cm:
              pending = None  # (b, u_ps, ypn) whose final dot is deferred
              for b in range(PC):
                lr = io.tile([NPAIR, 2, N], bf16)
                nc.sync.dma_start(out=lr, in_=lr_d[b])
                lhs = lr[:, 0, :]
                rhs0 = lr[:, 1, :]
                ypt = io.tile([128, NCHUNK, J], bf16)
                nc.sync.dma_start(out=ypt, in_=ypt_d[b])
                ypn = io.tile([J, N], f32)
                nc.sync.dma_start(out=ypn, in_=ypn_d[b])

                u_ps = pu.tile([128, N], f32)

                # Per chunk, one PSUM bank mutated in place:
                #   phase 1: eb = -e            -> min = -rowmax(e) -> smax
                #   phase 2: eb += -BIG on diag -> max = -rowmin_masked(e)
                #            -> tmin_neg
                # The two phases of a chunk form a PE->DVE->ACT->PE->DVE->ACT
                # chain; emitting phase2(c) after phase1(c+1) skews them so no
                # engine idles waiting for the same-chunk round trip.
                def phase1(c):
                    eb = pef.tile([128, N], f32, name="eb")
                    lsl = lhs[:, 128 * c:128 * (c + 1)]
                    nc.tensor.matmul(out=eb, lhsT=lsl, rhs=rhs0,
                                     start=True, stop=True)
                    rm = small.tile([128, 2], f32, name="rm")
                    nc.vector.tensor_reduce(
                        out=rm[:, 0:1], in_=eb, axis=AX.X, op=ALU.min,
                    )
                    # smax = Sign(e - rowmax): 0 at the argmax, -1 elsewhere
                    # (reads eb BEFORE the diagonal mutation in phase 2)
                    smax = masks.tile([128, N], bf16, name="smax")
                    nc.scalar.activation(out=smax, in_=eb, func=AF.Sign,
                                         bias=rm[:, 0:1], scale=-1.0)
                    return c, eb, rm, smax

                def phase2(c, eb, rm, smax):
                    # eb[diag] -= BIG  (self-distance excluded from the min;
                    # "stop" is sim-only bookkeeping, accumulating onto a
                    # closed group is fine on HW -> skip_group_check)
                    nc.tensor.matmul(
                        out=eb[:, 128 * c:128 * (c + 1)],
                        lhsT=neg_big_i, rhs=ident, start=False, stop=True,
                        skip_group_check=True,
                    )
                    nc.vector.tensor_reduce(
                        out=rm[:, 1:2], in_=eb, axis=AX.X, op=ALU.max,
                    )
                    # tmin_neg = Sign(rowmin - e): 0 at the argmin, +1 else
                    # (max(-e - BIGdiag) = -rowmin_masked(e); diag -> +1)
                    tmin_neg = masks.tile([128, N], bf16, name="tmin_neg")
                    nc.scalar.activation(out=tmin_neg, in_=eb, func=AF.Sign,
                                         bias=rm[:, 1:2], scale=-1.0)
                    # U += ypT_c^T @ (smax + tmin_neg)   (same stationary lhsT)
                    nc.tensor.matmul(out=u_ps, lhsT=ypt[:, c, :], rhs=smax,
                                     start=(c == 0), stop=False)
                    nc.tensor.matmul(out=u_ps, lhsT=ypt[:, c, :], rhs=tmin_neg,
                                     start=False, stop=(c == NCHUNK - 1))

                prev = None
                for c in range(NCHUNK):
                    if c == 3 and pending is not None:
                        # previous batch's final dot, overlapped mid-chunk-loop
                        final_dot(*pending)
                        pending = None
                    st = phase1(c)
                    if prev is not None:
                        phase2(*prev)
                    prev = st
                phase2(*prev)
                pending = (b, u_ps, ypn)
              final_dot(*pending)
            nc.sync.dma_start(out=out_d[:, 0:PC], in_=accs_v)
            nc.sync.dma_start(out=out_d[:, PC:2 * PC], in_=accs_a)

    nc.compile()
    _CACHE[key] = nc
    return nc


def _split3(a):
    """fp32 array -> (hi, mid, lo) bf16 parts with hi+mid+lo ~= a (~2^-27 rel)."""
    hi = a.astype(BF16)
    r = a - hi.astype(np.float32)
    mid = r.astype(BF16)
    lo = (r - mid.astype(np.float32)).astype(BF16)
    return hi, mid, lo


def _prep_inputs(ypred, xyz):
    """Host-side shard prep: slices, transposes, bf16 split operands."""
    yp = np.ascontiguousarray(ypred.reshape(BL, J, N).astype(np.float32, copy=False))
    x = xyz.reshape(BL, N, 3).astype(np.float32, copy=False)

    xt = np.ascontiguousarray(x.transpose(0, 2, 1))          # [BL, 3, N]
    ah, am, al = _split3(xt)                                  # bf16 [BL, 3, N]
    sq = np.einsum("bnd,bnd->bn", x, x).astype(np.float32)    # [BL, N]
    s1, s2, s3 = _split3(sq)

    # Split-product pairs kept for x[n,d]*x[m,d]: (lhs_part, rhs_part)
    pairs = [(0, 0), (0, 1), (0, 2), (1, 0), (1, 1), (2, 0)]
    parts = (ah, am, al)

    lhs = np.empty((BL, NPAIR, N), dtype=BF16)
    rhs0 = np.empty((BL, NPAIR, N), dtype=BF16)               # builds -e
    for d in range(3):
        for k, (i, jj) in enumerate(pairs):
            row = 6 * d + k
            lhs[:, row, :] = parts[i][:, d, :]
            rhs0[:, row, :] = (2.0 * parts[jj][:, d, :].astype(np.float32)).astype(BF16)
    lhs[:, 18, :] = np.ones((BL, N), dtype=BF16)
    lhs[:, 19, :] = np.ones((BL, N), dtype=BF16)
    lhs[:, 20, :] = np.ones((BL, N), dtype=BF16)
    rhs0[:, 18, :] = (-s1.astype(np.float32)).astype(BF16)
    rhs0[:, 19, :] = (-s2.astype(np.float32)).astype(BF16)
    rhs0[:, 20, :] = (-s3.astype(np.float32)).astype(BF16)

    # ypt[b, p, c, j] = yp[b, j, 128c+p]  (transposed logits, chunk-major)
    ypt = np.ascontiguousarray(
        yp.transpose(0, 2, 1).reshape(BL, NCHUNK, 128, J).transpose(0, 2, 1, 3)
    ).astype(BF16)

    in_maps = []
    for k in range(NCORES):
        s = slice(PC * k, PC * (k + 1))
        in_maps.append({
            "ypn": np.ascontiguousarray(yp[s]),
            "ypt": np.ascontiguousarray(ypt[s]),
            "lr": np.ascontiguousarray(np.stack([lhs[s], rhs0[s]], axis=2)),
        })
    return in_maps


def run(inputs, trace=False, trace_kwargs=None):
    """Run on 8 NeuronCores; returns (scalar np.float32 loss, BassKernelResults)."""
    from concourse.bass_utils import run_bass_kernel_spmd

    ypred = np.asarray(inputs["ypred"])
    xyz = np.asarray(inputs["xyz"])
    in_maps = _prep_inputs(ypred, xyz)
    nc = _build_nc()
    br = run_bass_kernel_spmd(
        nc, in_maps, core_ids=list(range(NCORES)),
        trace=trace, **(trace_kwargs or {}),
    )
    total = 0.0
    for r in br.results:
        total += float(r["out"].astype(np.float64).sum())
    loss = np.float32(total / (BL * N))
    return np.array(loss, dtype=np.float32), br


def kernel(ypred, xyz):
    out, _ = run({"ypred": ypred, "xyz": xyz})
    return out


if __name__ == "__main__":
    rng = np.random.default_rng(0)
    yp = rng.standard_normal((B, L, J, N), dtype=np.float32)
    xz = rng.standard_normal((B, L, N, 3), dtype=np.float32)
    print(kernel(yp, xz))

